# revision 8
# baseline (speedup 1.0000x reference)
"""Trainium2 Bass kernel for batched multi-head attention with key-padding mask.

Reference computation (per batch b, head h):
    scores = (Q @ K^T) / sqrt(64)               [S, S]
    scores = where(mask[b, k] == 0, -1e9)       (mask over keys)
    P      = softmax(scores, axis=-1)           [S, S]  (also an output)
    out    = P @ V                              [S, D]

Strategy (8 NeuronCores, batch*heads = 24 pairs -> 3 pairs/core):

Everything on-chip is computed in a TRANSPOSED layout so that the PE
contraction dim always sits on partitions and softmax bookkeeping is free:

  * S^T[k, q] = sum_d K^T[d,k] Q^T[d,q] with k on partitions.  The matmuls
    run in float32r (fp32 rounded to 11 mantissa bits, 4x faster than fp32
    on the PE).  To keep full fp32 precision on the scores, Q and K are
    split on the host into hi (11-bit) + lo (residual) parts and the
    product is computed in two accumulating passes:
        pass 1 (K=64):   Khi . Qhi
        pass 2 (K=128):  [Khi;Klo] . [Qlo;Qhi]  ( = Khi.Qlo + Klo.Qhi )
    dropping only the Klo.Qlo term (~2^-24 relative).  Measured on HW:
    2.1e-7 max rel err, same as native fp32 matmul.
  * The key-padding mask is applied via the ACT activation's per-partition
    bias: exp(S^T/8 + bias[k]) with bias[k] = -1e9 for masked keys.
  * ACT writes P_un^T directly in float32r (required so the PV matmul can
    consume it at full PE rate; costs ~2.4e-4 relative on p_attn/out).
  * V gets a ones-column appended on host ([S, 65]); the PV matmul
    out^T[c, q] = sum_k V'[k, c] P_un^T[k, q] accumulates over k-blocks in
    PSUM and its row 64 is the softmax denominator rowsum[q] for free.
  * rowsum -> reciprocal: bounced through DRAM to reshape [1,S] -> [128,S/128]
    (DVE reciprocal is ~8cyc/elem/lane; needs all 128 lanes), then the
    reciprocal row is broadcast to [128, S] with a stride-0 partition DMA.
  * DVE tensor_mul normalizes P_un^T in place; DMA writes P^T and out^T to
    HBM contiguously.  Host fixes the final layout with cheap swapaxes.
"""

import numpy as np

B, H, S, D = 2, 12, 2048, 64
NCORES = 8
PAIRS = (B * H) // NCORES  # 3 (b,h) pairs per core
VR = D + 1  # 65: V columns = 64 dims + 1 ones column (rowsum)
F32R_BITS = 11  # mantissa bits kept by the PE's float32r rounding (measured)

TRACE = False
LAST_EXEC_NS = None
LAST_RESULTS = None

_NC_CACHE = {}


def round_mant(x, bits=F32R_BITS):
    """Round fp32 to `bits` explicit mantissa bits (round-to-nearest-even).

    Matches TRN2's float32r rounding (verified on hardware for bits=11)."""
    x = np.ascontiguousarray(x, np.float32)
    xi = x.view(np.uint32)
    shift = 23 - bits
    unit = np.uint32(1 << shift)
    half = np.uint32(1 << (shift - 1))
    low = xi & np.uint32(unit - 1)
    xi2 = xi & ~np.uint32(unit - 1)
    rup = (low > half) | ((low == half) & ((xi2 >> np.uint32(shift)) & 1).astype(bool))
    xi2 = xi2 + np.where(rup, unit, np.uint32(0))
    return xi2.view(np.float32)


def emit_attention(nc, tc, ins, outs, pairs, s, d):
    """Emit the per-core attention program.

    ins:  qhc [pairs, 128, s]  rows 0..63 = Qlo^T, rows 64..127 = Qhi^T
          qhi [pairs, 64, s]   Qhi^T again at base partition 0 (pass-1 rhs;
                               matmul requires lhsT/rhs base partitions equal)
          khc [pairs, 128, s]  rows 0..63 = Khi^T, rows 64..127 = Klo^T
          vm  [pairs, s, 65]   V (pre-rounded to f32r) + ones column
          maskb [pairs, s]     additive mask bias (0 or -1e9) per key
          rs/rc [pairs, 1, s]  Internal DRAM scratch
    outs: pt [pairs, s, s] (= P^T, [k, q]) and ot [pairs, d, s] (= out^T).
    """
    import concourse.bass as bass
    import concourse.mybir as mybir
    from contextlib import ExitStack

    f32 = mybir.dt.float32
    f32r = mybir.dt.float32r
    vr = d + 1
    kb_n = s // 128  # k blocks
    qh_w = s // 2  # process q in two halves (PSUM budget)
    n512 = (qh_w + 511) // 512

    qhc_d, qhi_d, khc_d, vm_d = ins["qhc"], ins["qhi"], ins["khc"], ins["vm"]
    maskb_d = ins["maskb"]
    rs_d, rc_d = ins["rs"], ins["rc"]
    pt_d, ot_d = outs["pt"], outs["ot"]

    with ExitStack() as ctx:
        sb = ctx.enter_context(tc.tile_pool(name="sb", bufs=1))
        raw = ctx.enter_context(tc.tile_pool(name="raw", bufs=3))
        aux = ctx.enter_context(tc.tile_pool(name="aux", bufs=2))
        punt_pool = ctx.enter_context(tc.tile_pool(name="punt", bufs=kb_n))
        stp = ctx.enter_context(tc.tile_pool(name="stp", bufs=2, space="PSUM"))
        outp = ctx.enter_context(tc.tile_pool(name="outp", bufs=1, space="PSUM"))

        for p in range(pairs):
            # inputs arrive pre-rounded to f32r values; the round copies are
            # numerically identities but must exist as instructions: the BIR
            # verifier requires every matmul f32r operand to be produced by a
            # rounding-capable op (a DMA does not qualify, and an in-place
            # self-copy gets DCE'd).
            qraw = raw.tile([128, s], f32, tag="raw", name=f"qraw_{p}")
            nc.sync.dma_start(out=qraw, in_=qhc_d[p])
            qt = sb.tile([128, s], f32r, tag="qt", name=f"qt_{p}")
            nc.vector.tensor_copy(out=qt, in_=qraw)
            qhraw = raw.tile([64, s], f32, tag="raw", name=f"qhraw_{p}")
            nc.sync.dma_start(out=qhraw, in_=qhi_d[p])
            qht = sb.tile([64, s], f32r, tag="qhi", name=f"qhi_{p}")
            nc.vector.tensor_copy(out=qht, in_=qhraw)
            kraw = raw.tile([128, s], f32, tag="raw", name=f"kraw_{p}")
            nc.sync.dma_start(out=kraw, in_=khc_d[p])
            kt = sb.tile([128, s], f32r, tag="kt", name=f"kt_{p}")
            nc.vector.tensor_copy(out=kt, in_=kraw)
            vraw = raw.tile([128, kb_n, vr], f32, tag="raw", name=f"vraw_{p}")
            nc.sync.dma_start(
                out=vraw, in_=vm_d[p].rearrange("(n pp) c -> pp n c", pp=128)
            )
            vt = sb.tile([128, kb_n, vr], f32r, tag="v", name=f"v_{p}")
            nc.vector.tensor_copy(out=vt, in_=vraw)
            maskt = sb.tile([128, kb_n], f32, tag="mask", name=f"mask_{p}")
            nc.sync.dma_start(
                out=maskt, in_=maskb_d[p].rearrange("(n pp) -> pp n", pp=128)
            )

            punts = [
                punt_pool.tile([128, s], f32r, tag="punt", name=f"pun_{p}_{kb}")
                for kb in range(kb_n)
            ]
            outT = outp.tile([vr, s], f32, tag="outT", name=f"outT_{p}")

            for qh in range(2):
                for kb in range(kb_n):
                    st = stp.tile([128, qh_w], f32, tag="st", name=f"st_{p}_{qh}_{kb}")
                    ks = slice(kb * 128, (kb + 1) * 128)
                    for h2 in range(n512):
                        c0 = h2 * 512
                        c1 = min(qh_w, c0 + 512)
                        qs = slice(qh * qh_w + c0, qh * qh_w + c1)
                        nc.tensor.matmul(
                            st[:, c0:c1],
                            kt[0:64, ks],
                            qht[:, qs],
                            start=True,
                            stop=False,
                        )
                        nc.tensor.matmul(
                            st[:, c0:c1],
                            kt[:, ks],
                            qt[:, qs],
                            start=False,
                            stop=True,
                        )
                    nc.scalar.activation(
                        punts[kb][:, qh * qh_w : (qh + 1) * qh_w],
                        st[:, :],
                        mybir.ActivationFunctionType.Exp,
                        bias=maskt[:, kb : kb + 1],
                        scale=0.125,
                    )
                    for h2 in range(n512):
                        c0 = h2 * 512
                        c1 = min(qh_w, c0 + 512)
                        qs = slice(qh * qh_w + c0, qh * qh_w + c1)
                        nc.tensor.matmul(
                            outT[:, qs],
                            vt[:, kb, :],
                            punts[kb][:, qs],
                            start=(kb == 0),
                            stop=(kb == kb_n - 1),
                        )

            # --- softmax denominator: recip of rowsum (= outT row 64) ---
            rs_sb = aux.tile([1, s], f32, tag="aux", name=f"rs_{p}")
            nc.vector.tensor_copy(out=rs_sb, in_=outT[d : d + 1, :])
            nc.sync.dma_start(out=rs_d[p], in_=rs_sb)
            # reshape via DRAM so reciprocal runs on all 128 lanes
            rs128 = sb.tile([128, s // 128], f32, tag="rs128", name=f"rs128_{p}")
            nc.sync.dma_start(
                out=rs128, in_=rs_d[p].rearrange("a (pp c) -> (a pp) c", pp=128)
            )
            rec128 = sb.tile([128, s // 128], f32, tag="rec128", name=f"rec128_{p}")
            nc.vector.reciprocal(out=rec128, in_=rs128)
            nc.sync.dma_start(
                out=rc_d[p].rearrange("a (pp c) -> (a pp) c", pp=128), in_=rec128
            )
            # broadcast recip row across 128 partitions (stride-0 DRAM read)
            rb = sb.tile([128, s], f32, tag="rb", name=f"rb_{p}")
            rc_flat = rc_d[p]
            rb_src = bass.AP(
                tensor=rc_flat.tensor,
                offset=rc_flat.offset,
                ap=[[0, 128], rc_flat.ap[-1]],
            )
            nc.sync.dma_start(out=rb, in_=rb_src)

            # --- normalize + store out^T ---
            oTs = aux.tile([d, s], f32, tag="aux", name=f"oTs_{p}")
            nc.vector.tensor_copy(out=oTs, in_=outT[0:d, :])
            nc.vector.tensor_mul(out=oTs, in0=oTs, in1=rb[0:d, :])
            nc.sync.dma_start(out=ot_d[p], in_=oTs)

            # --- normalize + store P^T ---
            # the in-place multiply keeps f32r as its output dtype so the BIR
            # verifier still sees only rounding-capable writers of the tile
            # (PV matmuls consume it); costs one extra 11-bit rounding on the
            # stored p_attn.
            for kb in range(kb_n):
                pn = punts[kb]
                nc.vector.tensor_mul(out=pn, in0=pn, in1=rb)
                nc.sync.dma_start(
                    out=pt_d[p, kb * 128 : (kb + 1) * 128, :], in_=pn.bitcast(f32)
                )


def _build_nc(pairs=PAIRS, s=S, d=D):
    import concourse.bacc as bacc
    import concourse.mybir as mybir
    from concourse import tile

    key = (pairs, s, d)
    if key in _NC_CACHE:
        return _NC_CACHE[key]

    f32 = mybir.dt.float32
    vr = d + 1
    nc = bacc.Bacc(
        "TRN2",
        target_bir_lowering=False,
        debug=False,
        enable_asserts=False,
        num_devices=NCORES,
    )
    ins = {
        "qhc": nc.dram_tensor("qhc", [pairs, 128, s], f32, kind="ExternalInput").ap(),
        "qhi": nc.dram_tensor("qhi", [pairs, 64, s], f32, kind="ExternalInput").ap(),
        "khc": nc.dram_tensor("khc", [pairs, 128, s], f32, kind="ExternalInput").ap(),
        "vm": nc.dram_tensor("vm", [pairs, s, vr], f32, kind="ExternalInput").ap(),
        "maskb": nc.dram_tensor("maskb", [pairs, s], f32, kind="ExternalInput").ap(),
        "rs": nc.dram_tensor("rs", [pairs, 1, s], f32, kind="Internal").ap(),
        "rc": nc.dram_tensor("rc", [pairs, 1, s], f32, kind="Internal").ap(),
    }
    outs = {
        "pt": nc.dram_tensor("pt", [pairs, s, s], f32, kind="ExternalOutput").ap(),
        "ot": nc.dram_tensor("ot", [pairs, d, s], f32, kind="ExternalOutput").ap(),
    }
    with tile.TileContext(nc) as tc:
        emit_attention(nc, tc, ins, outs, pairs, s, d)
    nc.compile()
    _NC_CACHE[key] = nc
    return nc


def prep_inputs(query, key, value, mask):
    """Full inputs -> per-core in_maps (list of 8 dicts)."""
    q = np.ascontiguousarray(np.asarray(query, np.float32)).reshape(B * H, S, D)
    k = np.ascontiguousarray(np.asarray(key, np.float32)).reshape(B * H, S, D)
    v = np.ascontiguousarray(np.asarray(value, np.float32)).reshape(B * H, S, D)
    m = np.asarray(mask)

    qT = q.transpose(0, 2, 1)  # [24, 64, S]
    kT = k.transpose(0, 2, 1)
    qhi = round_mant(qT)
    qlo = (qT - qhi).astype(np.float32)
    khi = round_mant(kT)
    klo = (kT - khi).astype(np.float32)
    qhc = np.concatenate([qlo, qhi], axis=1)  # [24, 128, S]
    khc = np.concatenate([khi, klo], axis=1)  # [24, 128, S]
    vm = np.concatenate(
        [round_mant(v), np.ones((B * H, S, 1), np.float32)], axis=2
    )  # [24, S, 65]
    maskb = np.where(m == 0, np.float32(-1e9), np.float32(0.0)).astype(np.float32)
    maskb = np.repeat(maskb[:, None, :], H, axis=1).reshape(B * H, S)

    in_maps = []
    for c in range(NCORES):
        sl = slice(c * PAIRS, (c + 1) * PAIRS)
        in_maps.append(
            {
                "qhc": np.ascontiguousarray(qhc[sl]),
                "qhi": np.ascontiguousarray(qhi[sl]),
                "khc": np.ascontiguousarray(khc[sl]),
                "vm": np.ascontiguousarray(vm[sl]),
                "maskb": np.ascontiguousarray(maskb[sl]),
            }
        )
    return in_maps


def kernel(query, key, value, mask):
    global LAST_EXEC_NS, LAST_RESULTS
    from concourse.bass_utils import run_bass_kernel_spmd

    nc = _build_nc()
    in_maps = prep_inputs(query, key, value, mask)
    res = run_bass_kernel_spmd(
        nc,
        in_maps,
        core_ids=list(range(NCORES)),
        trace=TRACE,
    )
    LAST_EXEC_NS = res.exec_time_ns
    LAST_RESULTS = res
    pt = np.concatenate([r["pt"] for r in res.results], axis=0)  # [24, S(k), S(q)]
    ot = np.concatenate([r["ot"] for r in res.results], axis=0)  # [24, D, S]
    p_attn = np.ascontiguousarray(pt.reshape(B, H, S, S).swapaxes(2, 3))
    out = np.ascontiguousarray(ot.reshape(B, H, D, S).swapaxes(2, 3))
    return out, p_attn


# revision 12
# speedup vs baseline: 1.2412x; 1.2412x over previous
"""Trainium2 Bass kernel for batched multi-head attention with key-padding mask.

Reference computation (per batch b, head h):
    scores = (Q @ K^T) / sqrt(64)               [S, S]
    scores = where(mask[b, k] == 0, -1e9)       (mask over keys)
    P      = softmax(scores, axis=-1)           [S, S]  (also an output)
    out    = P @ V                              [S, D]

Strategy (8 NeuronCores, batch*heads = 24 pairs -> 3 pairs/core):

Everything on-chip is computed in a TRANSPOSED layout so that the PE
contraction dim always sits on partitions and softmax bookkeeping is free:

  * S^T[k, q] = sum_d K^T[d,k] Q^T[d,q] with k on partitions.  The matmuls
    run in float32r (fp32 rounded to 11 mantissa bits, 4x faster than fp32
    on the PE).  To keep full fp32 precision on the scores, Q and K are
    split on the host into hi (11-bit) + lo (residual) parts and the
    product is computed in two accumulating passes:
        pass 1 (K=64):   Khi . Qhi
        pass 2 (K=128):  [Khi;Klo] . [Qlo;Qhi]  ( = Khi.Qlo + Klo.Qhi )
    dropping only the Klo.Qlo term (~2^-24 relative).  Measured on HW:
    2.1e-7 max rel err, same as native fp32 matmul.
  * The key-padding mask is applied via the ACT activation's per-partition
    bias: exp(S^T/8 + bias[k]) with bias[k] = -1e9 for masked keys.
  * ACT writes P_un^T directly in float32r (required so the PV matmul can
    consume it at full PE rate; costs ~2.4e-4 relative on p_attn/out).
  * V gets a ones-column appended on host ([S, 65]); the PV matmul
    out^T[c, q] = sum_k V'[k, c] P_un^T[k, q] accumulates over k-blocks in
    PSUM and its row 64 is the softmax denominator rowsum[q] for free.
  * rowsum -> reciprocal: bounced through DRAM to reshape [1,S] -> [128,S/128]
    (DVE reciprocal is ~8cyc/elem/lane; needs all 128 lanes), then the
    reciprocal row is broadcast to [128, S] with a stride-0 partition DMA.
  * DVE tensor_mul normalizes P_un^T in place; DMA writes P^T and out^T to
    HBM contiguously.  Host fixes the final layout with cheap swapaxes.
"""

import numpy as np

B, H, S, D = 2, 12, 2048, 64
NCORES = 8
PAIRS = (B * H) // NCORES  # 3 (b,h) pairs per core
VR = D + 1  # 65: V columns = 64 dims + 1 ones column (rowsum)
F32R_BITS = 11  # mantissa bits kept by the PE's float32r rounding (measured)

TRACE = False
LAST_EXEC_NS = None
LAST_RESULTS = None

_NC_CACHE = {}


def round_mant(x, bits=F32R_BITS):
    """Round fp32 to `bits` explicit mantissa bits (round-to-nearest-even).

    Matches TRN2's float32r rounding (verified on hardware for bits=11)."""
    x = np.ascontiguousarray(x, np.float32)
    xi = x.view(np.uint32)
    shift = 23 - bits
    unit = np.uint32(1 << shift)
    half = np.uint32(1 << (shift - 1))
    low = xi & np.uint32(unit - 1)
    xi2 = xi & ~np.uint32(unit - 1)
    rup = (low > half) | ((low == half) & ((xi2 >> np.uint32(shift)) & 1).astype(bool))
    xi2 = xi2 + np.where(rup, unit, np.uint32(0))
    return xi2.view(np.float32)


def emit_attention(nc, tc, ins, outs, pairs, s, d):
    """Emit the per-core attention program.

    ins:  qhc [pairs, 128, s]  rows 0..63 = Qlo^T, rows 64..127 = Qhi^T
          qhi [pairs, 64, s]   Qhi^T again at base partition 0 (pass-1 rhs;
                               matmul requires lhsT/rhs base partitions equal)
          khc [pairs, 128, s]  rows 0..63 = Khi^T, rows 64..127 = Klo^T
          vm  [pairs, s, 65]   V (pre-rounded to f32r) + ones column
          maskb [pairs, s]     additive mask bias (0 or -1e9) per key
          rs/rc [pairs, 1, s]  Internal DRAM scratch
    outs: pt [pairs, s, s] (= P^T, [k, q]) and ot [pairs, d, s] (= out^T).
    """
    import concourse.bass as bass
    import concourse.mybir as mybir
    from contextlib import ExitStack

    f32 = mybir.dt.float32
    f32r = mybir.dt.float32r
    vr = d + 1
    kb_n = s // 128  # k blocks
    qh_w = s // 2  # process q in two halves (PSUM budget)
    n512 = (qh_w + 511) // 512

    qhc_d, qhi_d, khc_d, vm_d = ins["qhc"], ins["qhi"], ins["khc"], ins["vm"]
    maskb_d = ins["maskb"]
    rs_d, rc_d = ins["rs"], ins["rc"]
    pt_d, ot_d = outs["pt"], outs["ot"]

    with ExitStack() as ctx:
        sb = ctx.enter_context(tc.tile_pool(name="sb", bufs=1))
        raw = ctx.enter_context(tc.tile_pool(name="raw", bufs=1))
        aux = ctx.enter_context(tc.tile_pool(name="aux", bufs=2))
        punt_pool = ctx.enter_context(tc.tile_pool(name="punt", bufs=kb_n))
        stp = ctx.enter_context(tc.tile_pool(name="stp", bufs=2, space="PSUM"))
        outp = ctx.enter_context(tc.tile_pool(name="outp", bufs=1, space="PSUM"))

        def load_pair(p):
            # inputs arrive pre-rounded to f32r values; the round copies are
            # numerically identities but must exist as instructions: the BIR
            # verifier requires every matmul f32r operand to be produced by a
            # rounding-capable op (a DMA does not qualify, and an in-place
            # self-copy gets DCE'd).
            qraw = raw.tile([128, s], f32, tag="raw", name=f"qraw_{p}")
            nc.sync.dma_start(out=qraw, in_=qhc_d[p])
            qt = sb.tile([128, s], f32r, tag="qt", name=f"qt_{p}", bufs=2)
            nc.vector.tensor_copy(out=qt, in_=qraw)
            qhraw = raw.tile([64, s], f32, tag="raw", name=f"qhraw_{p}")
            nc.sync.dma_start(out=qhraw, in_=qhi_d[p])
            qht = sb.tile([64, s], f32r, tag="qhi", name=f"qhi_{p}", bufs=2)
            nc.vector.tensor_copy(out=qht, in_=qhraw)
            kraw = raw.tile([128, s], f32, tag="raw", name=f"kraw_{p}")
            nc.sync.dma_start(out=kraw, in_=khc_d[p])
            kt = sb.tile([128, s], f32r, tag="kt", name=f"kt_{p}", bufs=2)
            nc.vector.tensor_copy(out=kt, in_=kraw)
            vraw = raw.tile([128, kb_n, vr], f32, tag="raw", name=f"vraw_{p}")
            nc.sync.dma_start(
                out=vraw, in_=vm_d[p].rearrange("(n pp) c -> pp n c", pp=128)
            )
            vt = sb.tile([128, kb_n, vr], f32r, tag="v", name=f"v_{p}", bufs=1)
            nc.vector.tensor_copy(out=vt, in_=vraw)
            maskt = sb.tile([128, kb_n], f32, tag="mask", name=f"mask_{p}", bufs=2)
            nc.sync.dma_start(
                out=maskt, in_=maskb_d[p].rearrange("(n pp) -> pp n", pp=128)
            )
            return qt, qht, kt, vt, maskt

        loaded = {0: load_pair(0)}
        for p in range(pairs):
            if p + 1 < pairs:
                loaded[p + 1] = load_pair(p + 1)
            qt, qht, kt, vt, maskt = loaded.pop(p)

            punts = [
                punt_pool.tile([128, s], f32r, tag="punt", name=f"pun_{p}_{kb}")
                for kb in range(kb_n)
            ]
            outT = outp.tile([vr, s], f32, tag="outT", name=f"outT_{p}")

            for qh in range(2):
                qlo, qhi_ = qh * qh_w, (qh + 1) * qh_w
                for kb in range(kb_n):
                    st = stp.tile([128, qh_w], f32, tag="st", name=f"st_{p}_{qh}_{kb}")
                    ks = slice(kb * 128, (kb + 1) * 128)
                    chunks = [
                        (c0, min(qh_w, c0 + 512)) for c0 in range(0, qh_w, 512)
                    ]
                    # QK pass 1: Khi . Qhi (K=64)
                    for c0, c1 in chunks:
                        nc.tensor.matmul(
                            st[:, c0:c1],
                            kt[0:64, ks],
                            qht[:, qlo + c0 : qlo + c1],
                            start=True,
                            stop=False,
                        )
                    # QK pass 2: Khi.Qlo + Klo.Qhi (K=128)
                    for c0, c1 in chunks:
                        nc.tensor.matmul(
                            st[:, c0:c1],
                            kt[:, ks],
                            qt[:, qlo + c0 : qlo + c1],
                            start=False,
                            stop=True,
                        )
                    nc.scalar.activation(
                        punts[kb][:, qlo:qhi_],
                        st[:, :],
                        mybir.ActivationFunctionType.Exp,
                        bias=maskt[:, kb : kb + 1],
                        scale=0.125,
                    )
                    for c0, c1 in chunks:
                        nc.tensor.matmul(
                            outT[:, qlo + c0 : qlo + c1],
                            vt[:, kb, :],
                            punts[kb][:, qlo + c0 : qlo + c1],
                            start=(kb == 0),
                            stop=(kb == kb_n - 1),
                        )

                # --- per-q-half tail: recip of rowsum, normalize, store ---
                # (outT[:, qlo:qhi_] is complete once this qh's kb loop ends,
                # so the tail overlaps the other q-half's compute)
                rs_sb = aux.tile([1, qh_w], f32, tag="aux", name=f"rs_{p}_{qh}")
                nc.vector.tensor_copy(out=rs_sb, in_=outT[d : d + 1, qlo:qhi_])
                nc.sync.dma_start(out=rs_d[p, qh], in_=rs_sb)
                rs128 = sb.tile(
                    [128, qh_w // 128], f32, tag="rs128", name=f"rs128_{p}_{qh}"
                )
                nc.sync.dma_start(
                    out=rs128,
                    in_=rs_d[p, qh].rearrange("a (pp c) -> (a pp) c", pp=128),
                )
                rec128 = sb.tile(
                    [128, qh_w // 128], f32, tag="rec128", name=f"rec128_{p}_{qh}"
                )
                nc.vector.reciprocal(out=rec128, in_=rs128)
                nc.sync.dma_start(
                    out=rc_d[p, qh].rearrange("a (pp c) -> (a pp) c", pp=128),
                    in_=rec128,
                )
                # broadcast recip row across 128 partitions (stride-0 DRAM read)
                rb = sb.tile([128, qh_w], f32, tag="rb", name=f"rb_{p}_{qh}", bufs=1)
                rc_flat = rc_d[p, qh]
                rb_src = bass.AP(
                    tensor=rc_flat.tensor,
                    offset=rc_flat.offset,
                    ap=[[0, 128], rc_flat.ap[-1]],
                )
                nc.sync.dma_start(out=rb, in_=rb_src)

                oTs = aux.tile([d, qh_w], f32, tag="aux", name=f"oTs_{p}_{qh}")
                nc.vector.tensor_copy(out=oTs, in_=outT[0:d, qlo:qhi_])
                nc.vector.tensor_mul(out=oTs, in0=oTs, in1=rb[0:d, :])
                nc.sync.dma_start(out=ot_d[p, :, qlo:qhi_], in_=oTs)

                # normalize P^T in place; f32r output dtype keeps the BIR
                # verifier happy (PV matmuls consume this tile); costs one
                # extra 11-bit rounding on the stored p_attn.
                for kb in range(kb_n):
                    pn = punts[kb][:, qlo:qhi_]
                    nc.vector.tensor_mul(out=pn, in0=pn, in1=rb)
                    nc.sync.dma_start(
                        out=pt_d[p, kb * 128 : (kb + 1) * 128, qlo:qhi_],
                        in_=pn.bitcast(f32),
                    )


def _build_nc(pairs=PAIRS, s=S, d=D):
    import concourse.bacc as bacc
    import concourse.mybir as mybir
    from concourse import tile

    key = (pairs, s, d)
    if key in _NC_CACHE:
        return _NC_CACHE[key]

    f32 = mybir.dt.float32
    vr = d + 1
    nc = bacc.Bacc(
        "TRN2",
        target_bir_lowering=False,
        debug=False,
        enable_asserts=False,
        num_devices=NCORES,
    )
    ins = {
        "qhc": nc.dram_tensor("qhc", [pairs, 128, s], f32, kind="ExternalInput").ap(),
        "qhi": nc.dram_tensor("qhi", [pairs, 64, s], f32, kind="ExternalInput").ap(),
        "khc": nc.dram_tensor("khc", [pairs, 128, s], f32, kind="ExternalInput").ap(),
        "vm": nc.dram_tensor("vm", [pairs, s, vr], f32, kind="ExternalInput").ap(),
        "maskb": nc.dram_tensor("maskb", [pairs, s], f32, kind="ExternalInput").ap(),
        "rs": nc.dram_tensor("rs", [pairs, 2, 1, s // 2], f32, kind="Internal").ap(),
        "rc": nc.dram_tensor("rc", [pairs, 2, 1, s // 2], f32, kind="Internal").ap(),
    }
    outs = {
        "pt": nc.dram_tensor("pt", [pairs, s, s], f32, kind="ExternalOutput").ap(),
        "ot": nc.dram_tensor("ot", [pairs, d, s], f32, kind="ExternalOutput").ap(),
    }
    with tile.TileContext(nc) as tc:
        emit_attention(nc, tc, ins, outs, pairs, s, d)
    nc.compile()
    _NC_CACHE[key] = nc
    return nc


def prep_inputs(query, key, value, mask):
    """Full inputs -> per-core in_maps (list of 8 dicts)."""
    q = np.ascontiguousarray(np.asarray(query, np.float32)).reshape(B * H, S, D)
    k = np.ascontiguousarray(np.asarray(key, np.float32)).reshape(B * H, S, D)
    v = np.ascontiguousarray(np.asarray(value, np.float32)).reshape(B * H, S, D)
    m = np.asarray(mask)

    qT = q.transpose(0, 2, 1)  # [24, 64, S]
    kT = k.transpose(0, 2, 1)
    qhi = round_mant(qT)
    qlo = (qT - qhi).astype(np.float32)
    khi = round_mant(kT)
    klo = (kT - khi).astype(np.float32)
    qhc = np.concatenate([qlo, qhi], axis=1)  # [24, 128, S]
    khc = np.concatenate([khi, klo], axis=1)  # [24, 128, S]
    vm = np.concatenate(
        [round_mant(v), np.ones((B * H, S, 1), np.float32)], axis=2
    )  # [24, S, 65]
    maskb = np.where(m == 0, np.float32(-1e9), np.float32(0.0)).astype(np.float32)
    maskb = np.repeat(maskb[:, None, :], H, axis=1).reshape(B * H, S)

    in_maps = []
    for c in range(NCORES):
        sl = slice(c * PAIRS, (c + 1) * PAIRS)
        in_maps.append(
            {
                "qhc": np.ascontiguousarray(qhc[sl]),
                "qhi": np.ascontiguousarray(qhi[sl]),
                "khc": np.ascontiguousarray(khc[sl]),
                "vm": np.ascontiguousarray(vm[sl]),
                "maskb": np.ascontiguousarray(maskb[sl]),
            }
        )
    return in_maps


def kernel(query, key, value, mask):
    global LAST_EXEC_NS, LAST_RESULTS
    from concourse.bass_utils import run_bass_kernel_spmd

    nc = _build_nc()
    in_maps = prep_inputs(query, key, value, mask)
    res = run_bass_kernel_spmd(
        nc,
        in_maps,
        core_ids=list(range(NCORES)),
        trace=TRACE,
    )
    LAST_EXEC_NS = res.exec_time_ns
    LAST_RESULTS = res
    pt = np.concatenate([r["pt"] for r in res.results], axis=0)  # [24, S(k), S(q)]
    ot = np.concatenate([r["ot"] for r in res.results], axis=0)  # [24, D, S]
    p_attn = np.ascontiguousarray(pt.reshape(B, H, S, S).swapaxes(2, 3))
    out = np.ascontiguousarray(ot.reshape(B, H, D, S).swapaxes(2, 3))
    return out, p_attn


# revision 14
# speedup vs baseline: 1.2953x; 1.0436x over previous
"""Trainium2 Bass kernel for batched multi-head attention with key-padding mask.

Reference computation (per batch b, head h):
    scores = (Q @ K^T) / sqrt(64)               [S, S]
    scores = where(mask[b, k] == 0, -1e9)       (mask over keys)
    P      = softmax(scores, axis=-1)           [S, S]  (also an output)
    out    = P @ V                              [S, D]

Strategy (8 NeuronCores, batch*heads = 24 pairs -> 3 pairs/core):

Everything on-chip is computed in a TRANSPOSED layout so that the PE
contraction dim always sits on partitions and softmax bookkeeping is free:

  * S^T[k, q] = sum_d K^T[d,k] Q^T[d,q] with k on partitions.  The matmuls
    run in float32r (fp32 rounded to 11 mantissa bits, 4x faster than fp32
    on the PE).  To keep full fp32 precision on the scores, Q and K are
    split on the host into hi (11-bit) + lo (residual) parts and the
    product is computed in two accumulating passes:
        pass 1 (K=64):   Khi . Qhi
        pass 2 (K=128):  [Khi;Klo] . [Qlo;Qhi]  ( = Khi.Qlo + Klo.Qhi )
    dropping only the Klo.Qlo term (~2^-24 relative).  Measured on HW:
    2.1e-7 max rel err, same as native fp32 matmul.
  * The key-padding mask is applied via the ACT activation's per-partition
    bias: exp(S^T/8 + bias[k]) with bias[k] = -1e9 for masked keys.
  * ACT writes P_un^T directly in float32r (required so the PV matmul can
    consume it at full PE rate; costs ~2.4e-4 relative on p_attn/out).
  * V gets a ones-column appended on host ([S, 65]); the PV matmul
    out^T[c, q] = sum_k V'[k, c] P_un^T[k, q] accumulates over k-blocks in
    PSUM and its row 64 is the softmax denominator rowsum[q] for free.
  * rowsum -> reciprocal: bounced through DRAM to reshape [1,S] -> [128,S/128]
    (DVE reciprocal is ~8cyc/elem/lane; needs all 128 lanes), then the
    reciprocal row is broadcast to [128, S] with a stride-0 partition DMA.
  * DVE tensor_mul normalizes P_un^T in place; DMA writes P^T and out^T to
    HBM contiguously.  Host fixes the final layout with cheap swapaxes.
"""

import numpy as np

B, H, S, D = 2, 12, 2048, 64
NCORES = 8
PAIRS = (B * H) // NCORES  # 3 (b,h) pairs per core
VR = D + 1  # 65: V columns = 64 dims + 1 ones column (rowsum)
F32R_BITS = 11  # mantissa bits kept by the PE's float32r rounding (measured)

TRACE = False
LAST_EXEC_NS = None
LAST_RESULTS = None

_NC_CACHE = {}


def round_mant(x, bits=F32R_BITS):
    """Round fp32 to `bits` explicit mantissa bits (round-to-nearest-even).

    Matches TRN2's float32r rounding (verified on hardware for bits=11)."""
    x = np.ascontiguousarray(x, np.float32)
    xi = x.view(np.uint32)
    shift = 23 - bits
    unit = np.uint32(1 << shift)
    half = np.uint32(1 << (shift - 1))
    low = xi & np.uint32(unit - 1)
    xi2 = xi & ~np.uint32(unit - 1)
    rup = (low > half) | ((low == half) & ((xi2 >> np.uint32(shift)) & 1).astype(bool))
    xi2 = xi2 + np.where(rup, unit, np.uint32(0))
    return xi2.view(np.float32)


def emit_attention(nc, tc, ins, outs, pairs, s, d):
    """Emit the per-core attention program.

    ins:  qhc [pairs, 128, s]  rows 0..63 = Qlo^T, rows 64..127 = Qhi^T
          qhi [pairs, 64, s]   Qhi^T again at base partition 0 (pass-1 rhs;
                               matmul requires lhsT/rhs base partitions equal)
          khc [pairs, 128, s]  rows 0..63 = Khi^T, rows 64..127 = Klo^T
          vm  [pairs, s, 65]   V (pre-rounded to f32r) + ones column
          maskb [pairs, s]     additive mask bias (0 or -1e9) per key
    outs: pt [pairs, s, s] (= P^T, [k, q]) and ot [pairs, d, s] (= out^T).
    """
    import concourse.bass as bass
    import concourse.mybir as mybir
    from contextlib import ExitStack

    f32 = mybir.dt.float32
    f32r = mybir.dt.float32r
    vr = d + 1
    kb_n = s // 128  # k blocks
    qh_w = s // 2  # process q in two halves (PSUM budget)
    n512 = (qh_w + 511) // 512

    qhc_d, qhi_d, khc_d, vm_d = ins["qhc"], ins["qhi"], ins["khc"], ins["vm"]
    maskb_d = ins["maskb"]
    pt_d, ot_d = outs["pt"], outs["ot"]

    with ExitStack() as ctx:
        sb = ctx.enter_context(tc.tile_pool(name="sb", bufs=1))
        raw = ctx.enter_context(tc.tile_pool(name="raw", bufs=1))
        aux = ctx.enter_context(tc.tile_pool(name="aux", bufs=2))
        punt_pool = ctx.enter_context(tc.tile_pool(name="punt", bufs=kb_n))
        stp = ctx.enter_context(tc.tile_pool(name="stp", bufs=2, space="PSUM"))
        outp = ctx.enter_context(tc.tile_pool(name="outp", bufs=1, space="PSUM"))

        def load_pair(p):
            # inputs arrive pre-rounded to f32r values; the round copies are
            # numerically identities but must exist as instructions: the BIR
            # verifier requires every matmul f32r operand to be produced by a
            # rounding-capable op (a DMA does not qualify, and an in-place
            # self-copy gets DCE'd).
            qraw = raw.tile([128, s], f32, tag="raw", name=f"qraw_{p}")
            nc.sync.dma_start(out=qraw, in_=qhc_d[p])
            qt = sb.tile([128, s], f32r, tag="qt", name=f"qt_{p}", bufs=2)
            nc.gpsimd.tensor_copy(out=qt, in_=qraw)
            qhraw = raw.tile([64, s], f32, tag="raw", name=f"qhraw_{p}")
            nc.sync.dma_start(out=qhraw, in_=qhi_d[p])
            qht = sb.tile([64, s], f32r, tag="qhi", name=f"qhi_{p}", bufs=2)
            nc.gpsimd.tensor_copy(out=qht, in_=qhraw)
            kraw = raw.tile([128, s], f32, tag="raw", name=f"kraw_{p}")
            nc.sync.dma_start(out=kraw, in_=khc_d[p])
            kt = sb.tile([128, s], f32r, tag="kt", name=f"kt_{p}", bufs=2)
            nc.gpsimd.tensor_copy(out=kt, in_=kraw)
            vraw = raw.tile([128, kb_n, vr], f32, tag="raw", name=f"vraw_{p}")
            nc.sync.dma_start(
                out=vraw, in_=vm_d[p].rearrange("(n pp) c -> pp n c", pp=128)
            )
            vt = sb.tile([128, kb_n, vr], f32r, tag="v", name=f"v_{p}", bufs=1)
            nc.gpsimd.tensor_copy(out=vt, in_=vraw)
            maskt = sb.tile([128, kb_n], f32, tag="mask", name=f"mask_{p}", bufs=2)
            nc.sync.dma_start(
                out=maskt, in_=maskb_d[p].rearrange("(n pp) -> pp n", pp=128)
            )
            return qt, qht, kt, vt, maskt

        loaded = {0: load_pair(0)}
        for p in range(pairs):
            if p + 1 < pairs:
                loaded[p + 1] = load_pair(p + 1)
            qt, qht, kt, vt, maskt = loaded.pop(p)

            punts = [
                punt_pool.tile([128, s], f32r, tag="punt", name=f"pun_{p}_{kb}")
                for kb in range(kb_n)
            ]
            outT = outp.tile([vr, s], f32, tag="outT", name=f"outT_{p}")

            for qh in range(2):
                qlo, qhi_ = qh * qh_w, (qh + 1) * qh_w
                for kb in range(kb_n):
                    st = stp.tile([128, qh_w], f32, tag="st", name=f"st_{p}_{qh}_{kb}")
                    ks = slice(kb * 128, (kb + 1) * 128)
                    chunks = [
                        (c0, min(qh_w, c0 + 512)) for c0 in range(0, qh_w, 512)
                    ]
                    # QK pass 1: Khi . Qhi (K=64)
                    for c0, c1 in chunks:
                        nc.tensor.matmul(
                            st[:, c0:c1],
                            kt[0:64, ks],
                            qht[:, qlo + c0 : qlo + c1],
                            start=True,
                            stop=False,
                        )
                    # QK pass 2: Khi.Qlo + Klo.Qhi (K=128)
                    for c0, c1 in chunks:
                        nc.tensor.matmul(
                            st[:, c0:c1],
                            kt[:, ks],
                            qt[:, qlo + c0 : qlo + c1],
                            start=False,
                            stop=True,
                        )
                    nc.scalar.activation(
                        punts[kb][:, qlo:qhi_],
                        st[:, :],
                        mybir.ActivationFunctionType.Exp,
                        bias=maskt[:, kb : kb + 1],
                        scale=0.125,
                    )
                    for c0, c1 in chunks:
                        nc.tensor.matmul(
                            outT[:, qlo + c0 : qlo + c1],
                            vt[:, kb, :],
                            punts[kb][:, qlo + c0 : qlo + c1],
                            start=(kb == 0),
                            stop=(kb == kb_n - 1),
                        )

                # --- per-q-half tail: recip of rowsum, normalize, store ---
                # (outT[:, qlo:qhi_] is complete once this qh's kb loop ends,
                # so the tail overlaps the other q-half's compute)
                # reciprocal of rowsum, entirely in SBUF:
                #   [1, qh_w] row -> [128, qh_w/128] (SBUF-SBUF DMA reshape,
                #   so DVE reciprocal runs on all 128 lanes) -> back to a
                #   [1, qh_w] row -> stride-0 partition-replicate to [128, qh_w]
                rs_sb = aux.tile([1, qh_w], f32, tag="aux", name=f"rs_{p}_{qh}")
                nc.vector.tensor_copy(out=rs_sb, in_=outT[d : d + 1, qlo:qhi_])
                rs128 = sb.tile(
                    [128, qh_w // 128], f32, tag="rs128", name=f"rs128_{p}_{qh}"
                )
                rs_split = bass.AP(
                    tensor=rs_sb.tensor,
                    offset=rs_sb.offset,
                    ap=[[1, 1], [qh_w // 128, 128], [1, qh_w // 128]],
                )
                nc.sync.dma_start(out=rs128, in_=rs_split)
                rec128 = sb.tile(
                    [128, qh_w // 128], f32, tag="rec128", name=f"rec128_{p}_{qh}"
                )
                nc.vector.reciprocal(out=rec128, in_=rs128)
                rec_row = aux.tile([1, qh_w], f32, tag="aux", name=f"rr_{p}_{qh}")
                rr_split = bass.AP(
                    tensor=rec_row.tensor,
                    offset=rec_row.offset,
                    ap=[[1, 1], [qh_w // 128, 128], [1, qh_w // 128]],
                )
                nc.sync.dma_start(out=rr_split, in_=rec128)
                rb = sb.tile([128, qh_w], f32, tag="rb", name=f"rb_{p}_{qh}", bufs=1)
                nc.gpsimd.partition_broadcast(rb, rec_row)

                oTs = aux.tile([d, qh_w], f32, tag="aux", name=f"oTs_{p}_{qh}")
                nc.vector.tensor_copy(out=oTs, in_=outT[0:d, qlo:qhi_])
                nc.vector.tensor_mul(out=oTs, in0=oTs, in1=rb[0:d, :])
                nc.sync.dma_start(out=ot_d[p, :, qlo:qhi_], in_=oTs)

                # normalize P^T in place; f32r output dtype keeps the BIR
                # verifier happy (PV matmuls consume this tile); costs one
                # extra 11-bit rounding on the stored p_attn.
                for kb in range(kb_n):
                    pn = punts[kb][:, qlo:qhi_]
                    nc.vector.tensor_mul(out=pn, in0=pn, in1=rb)
                    nc.sync.dma_start(
                        out=pt_d[p, kb * 128 : (kb + 1) * 128, qlo:qhi_],
                        in_=pn.bitcast(f32),
                    )


def _build_nc(pairs=PAIRS, s=S, d=D):
    import concourse.bacc as bacc
    import concourse.mybir as mybir
    from concourse import tile

    key = (pairs, s, d)
    if key in _NC_CACHE:
        return _NC_CACHE[key]

    f32 = mybir.dt.float32
    vr = d + 1
    nc = bacc.Bacc(
        "TRN2",
        target_bir_lowering=False,
        debug=False,
        enable_asserts=False,
        num_devices=NCORES,
    )
    ins = {
        "qhc": nc.dram_tensor("qhc", [pairs, 128, s], f32, kind="ExternalInput").ap(),
        "qhi": nc.dram_tensor("qhi", [pairs, 64, s], f32, kind="ExternalInput").ap(),
        "khc": nc.dram_tensor("khc", [pairs, 128, s], f32, kind="ExternalInput").ap(),
        "vm": nc.dram_tensor("vm", [pairs, s, vr], f32, kind="ExternalInput").ap(),
        "maskb": nc.dram_tensor("maskb", [pairs, s], f32, kind="ExternalInput").ap(),
    }
    outs = {
        "pt": nc.dram_tensor("pt", [pairs, s, s], f32, kind="ExternalOutput").ap(),
        "ot": nc.dram_tensor("ot", [pairs, d, s], f32, kind="ExternalOutput").ap(),
    }
    with tile.TileContext(nc) as tc:
        emit_attention(nc, tc, ins, outs, pairs, s, d)
    nc.compile()
    _NC_CACHE[key] = nc
    return nc


def prep_inputs(query, key, value, mask):
    """Full inputs -> per-core in_maps (list of 8 dicts)."""
    q = np.ascontiguousarray(np.asarray(query, np.float32)).reshape(B * H, S, D)
    k = np.ascontiguousarray(np.asarray(key, np.float32)).reshape(B * H, S, D)
    v = np.ascontiguousarray(np.asarray(value, np.float32)).reshape(B * H, S, D)
    m = np.asarray(mask)

    qT = q.transpose(0, 2, 1)  # [24, 64, S]
    kT = k.transpose(0, 2, 1)
    qhi = round_mant(qT)
    qlo = (qT - qhi).astype(np.float32)
    khi = round_mant(kT)
    klo = (kT - khi).astype(np.float32)
    qhc = np.concatenate([qlo, qhi], axis=1)  # [24, 128, S]
    khc = np.concatenate([khi, klo], axis=1)  # [24, 128, S]
    vm = np.concatenate(
        [round_mant(v), np.ones((B * H, S, 1), np.float32)], axis=2
    )  # [24, S, 65]
    maskb = np.where(m == 0, np.float32(-1e9), np.float32(0.0)).astype(np.float32)
    maskb = np.repeat(maskb[:, None, :], H, axis=1).reshape(B * H, S)

    in_maps = []
    for c in range(NCORES):
        sl = slice(c * PAIRS, (c + 1) * PAIRS)
        in_maps.append(
            {
                "qhc": np.ascontiguousarray(qhc[sl]),
                "qhi": np.ascontiguousarray(qhi[sl]),
                "khc": np.ascontiguousarray(khc[sl]),
                "vm": np.ascontiguousarray(vm[sl]),
                "maskb": np.ascontiguousarray(maskb[sl]),
            }
        )
    return in_maps


def kernel(query, key, value, mask):
    global LAST_EXEC_NS, LAST_RESULTS
    from concourse.bass_utils import run_bass_kernel_spmd

    nc = _build_nc()
    in_maps = prep_inputs(query, key, value, mask)
    res = run_bass_kernel_spmd(
        nc,
        in_maps,
        core_ids=list(range(NCORES)),
        trace=TRACE,
    )
    LAST_EXEC_NS = res.exec_time_ns
    LAST_RESULTS = res
    pt = np.concatenate([r["pt"] for r in res.results], axis=0)  # [24, S(k), S(q)]
    ot = np.concatenate([r["ot"] for r in res.results], axis=0)  # [24, D, S]
    p_attn = np.ascontiguousarray(pt.reshape(B, H, S, S).swapaxes(2, 3))
    out = np.ascontiguousarray(ot.reshape(B, H, D, S).swapaxes(2, 3))
    return out, p_attn


# revision 15
# speedup vs baseline: 1.8864x; 1.4563x over previous
"""Trainium2 Bass kernel for batched multi-head attention with key-padding mask.

Reference computation (per batch b, head h):
    scores = (Q @ K^T) / sqrt(64)               [S, S]
    scores = where(mask[b, k] == 0, -1e9)       (mask over keys)
    P      = softmax(scores, axis=-1)           [S, S]  (also an output)
    out    = P @ V                              [S, D]

Strategy (8 NeuronCores, batch*heads = 24 pairs -> 3 pairs/core):

Everything on-chip is computed in a TRANSPOSED layout so that the PE
contraction dim always sits on partitions and softmax bookkeeping is free:

  * S^T[k, q] = sum_d K^T[d,k] Q^T[d,q] with k on partitions.  The matmuls
    run in float32r (fp32 rounded to 11 mantissa bits, 4x faster than fp32
    on the PE).  To keep full fp32 precision on the scores, Q and K are
    split on the host into hi (11-bit) + lo (residual) parts and the
    product is computed in two accumulating passes:
        pass 1 (K=64):   Khi . Qhi
        pass 2 (K=128):  [Khi;Klo] . [Qlo;Qhi]  ( = Khi.Qlo + Klo.Qhi )
    dropping only the Klo.Qlo term (~2^-24 relative).  Measured on HW:
    2.1e-7 max rel err, same as native fp32 matmul.
  * The key-padding mask is applied via the ACT activation's per-partition
    bias: exp(S^T/8 + bias[k]) with bias[k] = -1e9 for masked keys.
  * ACT writes P_un^T directly in float32r (required so the PV matmul can
    consume it at full PE rate; costs ~2.4e-4 relative on p_attn/out).
  * V gets a ones-column appended on host ([S, 65]); the PV matmul
    out^T[c, q] = sum_k V'[k, c] P_un^T[k, q] accumulates over k-blocks in
    PSUM and its row 64 is the softmax denominator rowsum[q] for free.
  * rowsum -> reciprocal: bounced through DRAM to reshape [1,S] -> [128,S/128]
    (DVE reciprocal is ~8cyc/elem/lane; needs all 128 lanes), then the
    reciprocal row is broadcast to [128, S] with a stride-0 partition DMA.
  * DVE tensor_mul normalizes P_un^T in place; DMA writes P^T and out^T to
    HBM contiguously.  Host fixes the final layout with cheap swapaxes.
"""

import numpy as np

B, H, S, D = 2, 12, 2048, 64
NCORES = 8
PAIRS = (B * H) // NCORES  # 3 (b,h) pairs per core
VR = D + 1  # 65: V columns = 64 dims + 1 ones column (rowsum)
F32R_BITS = 11  # mantissa bits kept by the PE's float32r rounding (measured)

TRACE = False
LAST_EXEC_NS = None
LAST_RESULTS = None

_NC_CACHE = {}


def round_mant(x, bits=F32R_BITS):
    """Round fp32 to `bits` explicit mantissa bits (round-to-nearest-even).

    Matches TRN2's float32r rounding (verified on hardware for bits=11)."""
    x = np.ascontiguousarray(x, np.float32)
    xi = x.view(np.uint32)
    shift = 23 - bits
    unit = np.uint32(1 << shift)
    half = np.uint32(1 << (shift - 1))
    low = xi & np.uint32(unit - 1)
    xi2 = xi & ~np.uint32(unit - 1)
    rup = (low > half) | ((low == half) & ((xi2 >> np.uint32(shift)) & 1).astype(bool))
    xi2 = xi2 + np.where(rup, unit, np.uint32(0))
    return xi2.view(np.float32)


def emit_attention(nc, tc, ins, outs, pairs, s, d):
    """Emit the per-core attention program.

    ins:  qhc [pairs, 128, s]  bf16, rows 0..63 = Qlo^T, rows 64..127 = Qhi^T
          qhi [pairs, 64, s]   bf16 Qhi^T again at base partition 0 (pass-1
                               rhs; matmul requires equal base partitions)
          khc [pairs, 128, s]  bf16, rows 0..63 = Khi^T, rows 64..127 = Klo^T
          vm  [pairs, s, 65]   V (pre-rounded to f32r) + ones column
          maskb [pairs, s]     additive mask bias (0 or -1e9) per key
    outs: pt [pairs, s, s] (= P^T, [k, q]) and ot [pairs, d, s] (= out^T).
    """
    import concourse.bass as bass
    import concourse.mybir as mybir
    from contextlib import ExitStack

    f32 = mybir.dt.float32
    f32r = mybir.dt.float32r
    bf16 = mybir.dt.bfloat16
    vr = d + 1
    kb_n = s // 128  # k blocks
    qh_w = s // 2  # process q in two halves (PSUM budget)
    n512 = (qh_w + 511) // 512

    qhc_d, qhi_d, khc_d, vm_d = ins["qhc"], ins["qhi"], ins["khc"], ins["vm"]
    maskb_d = ins["maskb"]
    pt_d, ot_d = outs["pt"], outs["ot"]

    with ExitStack() as ctx:
        sb = ctx.enter_context(tc.tile_pool(name="sb", bufs=1))
        raw = ctx.enter_context(tc.tile_pool(name="raw", bufs=1))
        aux = ctx.enter_context(tc.tile_pool(name="aux", bufs=2))
        punt_pool = ctx.enter_context(tc.tile_pool(name="punt", bufs=kb_n + 1))
        stp = ctx.enter_context(tc.tile_pool(name="stp", bufs=2, space="PSUM"))
        outp = ctx.enter_context(tc.tile_pool(name="outp", bufs=1, space="PSUM"))

        def load_pair(p):
            # Q/K arrive as bf16 hi/lo stacks -> straight DMA, no rounding op
            # needed (the f32r verifier rule applies only to f32r operands).
            qt = sb.tile([128, s], bf16, tag="qt", name=f"qt_{p}", bufs=2)
            nc.sync.dma_start(out=qt, in_=qhc_d[p])
            qht = sb.tile([64, s], bf16, tag="qhi", name=f"qhi_{p}", bufs=2)
            nc.sync.dma_start(out=qht, in_=qhi_d[p])
            kt = sb.tile([128, s], bf16, tag="kt", name=f"kt_{p}", bufs=2)
            nc.sync.dma_start(out=kt, in_=khc_d[p])
            # V arrives pre-rounded to f32r values; the round copy is a
            # numeric identity but must exist: the BIR verifier requires every
            # f32r matmul operand to be produced by a rounding-capable op.
            vraw = raw.tile([128, kb_n, vr], f32, tag="raw", name=f"vraw_{p}")
            nc.sync.dma_start(
                out=vraw, in_=vm_d[p].rearrange("(n pp) c -> pp n c", pp=128)
            )
            vt = sb.tile([128, kb_n, vr], f32r, tag="v", name=f"v_{p}", bufs=1)
            nc.gpsimd.tensor_copy(out=vt, in_=vraw)
            maskt = sb.tile([128, kb_n], f32, tag="mask", name=f"mask_{p}", bufs=2)
            nc.sync.dma_start(
                out=maskt, in_=maskb_d[p].rearrange("(n pp) -> pp n", pp=128)
            )
            return qt, qht, kt, vt, maskt

        loaded = {0: load_pair(0)}
        for p in range(pairs):
            if p + 1 < pairs:
                loaded[p + 1] = load_pair(p + 1)
            qt, qht, kt, vt, maskt = loaded.pop(p)

            punts = [
                punt_pool.tile([128, s], f32r, tag="punt", name=f"pun_{p}_{kb}")
                for kb in range(kb_n)
            ]
            outT = outp.tile([vr, s], f32, tag="outT", name=f"outT_{p}")

            for qh in range(2):
                qlo, qhi_ = qh * qh_w, (qh + 1) * qh_w
                for kb in range(kb_n):
                    st = stp.tile([128, qh_w], f32, tag="st", name=f"st_{p}_{qh}_{kb}")
                    ks = slice(kb * 128, (kb + 1) * 128)
                    chunks = [
                        (c0, min(qh_w, c0 + 512)) for c0 in range(0, qh_w, 512)
                    ]
                    # QK pass 1: Khi . Qhi (K=64)
                    for c0, c1 in chunks:
                        nc.tensor.matmul(
                            st[:, c0:c1],
                            kt[0:64, ks],
                            qht[:, qlo + c0 : qlo + c1],
                            start=True,
                            stop=False,
                        )
                    # QK pass 2: Khi.Qlo + Klo.Qhi (K=128)
                    for c0, c1 in chunks:
                        nc.tensor.matmul(
                            st[:, c0:c1],
                            kt[:, ks],
                            qt[:, qlo + c0 : qlo + c1],
                            start=False,
                            stop=True,
                        )
                    nc.scalar.activation(
                        punts[kb][:, qlo:qhi_],
                        st[:, :],
                        mybir.ActivationFunctionType.Exp,
                        bias=maskt[:, kb : kb + 1],
                        scale=0.125,
                    )
                    for c0, c1 in chunks:
                        nc.tensor.matmul(
                            outT[:, qlo + c0 : qlo + c1],
                            vt[:, kb, :],
                            punts[kb][:, qlo + c0 : qlo + c1],
                            start=(kb == 0),
                            stop=(kb == kb_n - 1),
                        )

                # --- per-q-half tail: recip of rowsum, normalize, store ---
                # (outT[:, qlo:qhi_] is complete once this qh's kb loop ends,
                # so the tail overlaps the other q-half's compute)
                # reciprocal of rowsum, entirely in SBUF:
                #   [1, qh_w] row -> [128, qh_w/128] (SBUF-SBUF DMA reshape,
                #   so DVE reciprocal runs on all 128 lanes) -> back to a
                #   [1, qh_w] row -> stride-0 partition-replicate to [128, qh_w]
                rs_sb = aux.tile([1, qh_w], f32, tag="aux", name=f"rs_{p}_{qh}")
                nc.vector.tensor_copy(out=rs_sb, in_=outT[d : d + 1, qlo:qhi_])
                rs128 = sb.tile(
                    [128, qh_w // 128], f32, tag="rs128", name=f"rs128_{p}_{qh}"
                )
                rs_split = bass.AP(
                    tensor=rs_sb.tensor,
                    offset=rs_sb.offset,
                    ap=[[1, 1], [qh_w // 128, 128], [1, qh_w // 128]],
                )
                nc.sync.dma_start(out=rs128, in_=rs_split)
                rec128 = sb.tile(
                    [128, qh_w // 128], f32, tag="rec128", name=f"rec128_{p}_{qh}"
                )
                nc.vector.reciprocal(out=rec128, in_=rs128)
                rec_row = aux.tile([1, qh_w], f32, tag="aux", name=f"rr_{p}_{qh}")
                rr_split = bass.AP(
                    tensor=rec_row.tensor,
                    offset=rec_row.offset,
                    ap=[[1, 1], [qh_w // 128, 128], [1, qh_w // 128]],
                )
                nc.sync.dma_start(out=rr_split, in_=rec128)
                rb = sb.tile([128, qh_w], f32, tag="rb", name=f"rb_{p}_{qh}", bufs=2)
                nc.gpsimd.partition_broadcast(rb, rec_row)

                oTs = aux.tile([d, qh_w], f32, tag="aux", name=f"oTs_{p}_{qh}")
                nc.vector.tensor_copy(out=oTs, in_=outT[0:d, qlo:qhi_])
                nc.vector.tensor_mul(out=oTs, in0=oTs, in1=rb[0:d, :])
                nc.sync.dma_start(out=ot_d[p, :, qlo:qhi_], in_=oTs)

                # normalize P^T in place; f32r output dtype keeps the BIR
                # verifier happy (PV matmuls consume this tile); costs one
                # extra 11-bit rounding on the stored p_attn.
                for kb in range(kb_n):
                    pn = punts[kb][:, qlo:qhi_]
                    nc.vector.tensor_mul(out=pn, in0=pn, in1=rb)
                    nc.sync.dma_start(
                        out=pt_d[p, kb * 128 : (kb + 1) * 128, qlo:qhi_],
                        in_=pn.bitcast(f32),
                    )


def _build_nc(pairs=PAIRS, s=S, d=D):
    import concourse.bacc as bacc
    import concourse.mybir as mybir
    from concourse import tile

    key = (pairs, s, d)
    if key in _NC_CACHE:
        return _NC_CACHE[key]

    f32 = mybir.dt.float32
    vr = d + 1
    nc = bacc.Bacc(
        "TRN2",
        target_bir_lowering=False,
        debug=False,
        enable_asserts=False,
        num_devices=NCORES,
    )
    bf16 = mybir.dt.bfloat16
    ins = {
        "qhc": nc.dram_tensor("qhc", [pairs, 128, s], bf16, kind="ExternalInput").ap(),
        "qhi": nc.dram_tensor("qhi", [pairs, 64, s], bf16, kind="ExternalInput").ap(),
        "khc": nc.dram_tensor("khc", [pairs, 128, s], bf16, kind="ExternalInput").ap(),
        "vm": nc.dram_tensor("vm", [pairs, s, vr], f32, kind="ExternalInput").ap(),
        "maskb": nc.dram_tensor("maskb", [pairs, s], f32, kind="ExternalInput").ap(),
    }
    outs = {
        "pt": nc.dram_tensor("pt", [pairs, s, s], f32, kind="ExternalOutput").ap(),
        "ot": nc.dram_tensor("ot", [pairs, d, s], f32, kind="ExternalOutput").ap(),
    }
    with tile.TileContext(nc) as tc:
        emit_attention(nc, tc, ins, outs, pairs, s, d)
    nc.compile()
    _NC_CACHE[key] = nc
    return nc


def prep_inputs(query, key, value, mask):
    """Full inputs -> per-core in_maps (list of 8 dicts)."""
    q = np.ascontiguousarray(np.asarray(query, np.float32)).reshape(B * H, S, D)
    k = np.ascontiguousarray(np.asarray(key, np.float32)).reshape(B * H, S, D)
    v = np.ascontiguousarray(np.asarray(value, np.float32)).reshape(B * H, S, D)
    m = np.asarray(mask)

    import ml_dtypes

    bf16 = ml_dtypes.bfloat16
    qT = q.transpose(0, 2, 1)  # [24, 64, S]
    kT = k.transpose(0, 2, 1)
    qhi = qT.astype(bf16)
    qlo = (qT - qhi.astype(np.float32)).astype(bf16)
    khi = kT.astype(bf16)
    klo = (kT - khi.astype(np.float32)).astype(bf16)
    qhc = np.concatenate([qlo, qhi], axis=1)  # [24, 128, S] bf16
    khc = np.concatenate([khi, klo], axis=1)  # [24, 128, S] bf16
    vm = np.concatenate(
        [round_mant(v), np.ones((B * H, S, 1), np.float32)], axis=2
    )  # [24, S, 65]
    maskb = np.where(m == 0, np.float32(-1e9), np.float32(0.0)).astype(np.float32)
    maskb = np.repeat(maskb[:, None, :], H, axis=1).reshape(B * H, S)

    in_maps = []
    for c in range(NCORES):
        sl = slice(c * PAIRS, (c + 1) * PAIRS)
        in_maps.append(
            {
                "qhc": np.ascontiguousarray(qhc[sl]),
                "qhi": np.ascontiguousarray(qhi[sl]),
                "khc": np.ascontiguousarray(khc[sl]),
                "vm": np.ascontiguousarray(vm[sl]),
                "maskb": np.ascontiguousarray(maskb[sl]),
            }
        )
    return in_maps


def kernel(query, key, value, mask):
    global LAST_EXEC_NS, LAST_RESULTS
    from concourse.bass_utils import run_bass_kernel_spmd

    nc = _build_nc()
    in_maps = prep_inputs(query, key, value, mask)
    res = run_bass_kernel_spmd(
        nc,
        in_maps,
        core_ids=list(range(NCORES)),
        trace=TRACE,
    )
    LAST_EXEC_NS = res.exec_time_ns
    LAST_RESULTS = res
    pt = np.concatenate([r["pt"] for r in res.results], axis=0)  # [24, S(k), S(q)]
    ot = np.concatenate([r["ot"] for r in res.results], axis=0)  # [24, D, S]
    p_attn = np.ascontiguousarray(pt.reshape(B, H, S, S).swapaxes(2, 3))
    out = np.ascontiguousarray(ot.reshape(B, H, D, S).swapaxes(2, 3))
    return out, p_attn


# revision 16
# speedup vs baseline: 2.0974x; 1.1119x over previous
"""Trainium2 Bass kernel for batched multi-head attention with key-padding mask.

Reference computation (per batch b, head h):
    scores = (Q @ K^T) / sqrt(64)               [S, S]
    scores = where(mask[b, k] == 0, -1e9)       (mask over keys)
    P      = softmax(scores, axis=-1)           [S, S]  (also an output)
    out    = P @ V                              [S, D]

Strategy (8 NeuronCores, batch*heads = 24 pairs -> 3 pairs/core):

Everything on-chip is computed in a TRANSPOSED layout so that the PE
contraction dim always sits on partitions and softmax bookkeeping is free:

  * S^T[k, q] = sum_d K^T[d,k] Q^T[d,q] with k on partitions.  The matmuls
    run in float32r (fp32 rounded to 11 mantissa bits, 4x faster than fp32
    on the PE).  To keep full fp32 precision on the scores, Q and K are
    split on the host into hi (11-bit) + lo (residual) parts and the
    product is computed in two accumulating passes:
        pass 1 (K=64):   Khi . Qhi
        pass 2 (K=128):  [Khi;Klo] . [Qlo;Qhi]  ( = Khi.Qlo + Klo.Qhi )
    dropping only the Klo.Qlo term (~2^-24 relative).  Measured on HW:
    2.1e-7 max rel err, same as native fp32 matmul.
  * The key-padding mask is applied via the ACT activation's per-partition
    bias: exp(S^T/8 + bias[k]) with bias[k] = -1e9 for masked keys.
  * ACT writes P_un^T directly in float32r (required so the PV matmul can
    consume it at full PE rate; costs ~2.4e-4 relative on p_attn/out).
  * V gets a ones-column appended on host ([S, 65]); the PV matmul
    out^T[c, q] = sum_k V'[k, c] P_un^T[k, q] accumulates over k-blocks in
    PSUM and its row 64 is the softmax denominator rowsum[q] for free.
  * rowsum -> reciprocal: bounced through DRAM to reshape [1,S] -> [128,S/128]
    (DVE reciprocal is ~8cyc/elem/lane; needs all 128 lanes), then the
    reciprocal row is broadcast to [128, S] with a stride-0 partition DMA.
  * DVE tensor_mul normalizes P_un^T in place; DMA writes P^T and out^T to
    HBM contiguously.  Host fixes the final layout with cheap swapaxes.
"""

import numpy as np

B, H, S, D = 2, 12, 2048, 64
NCORES = 8
PAIRS = (B * H) // NCORES  # 3 (b,h) pairs per core
VR = D + 1  # 65: V columns = 64 dims + 1 ones column (rowsum)
F32R_BITS = 11  # mantissa bits kept by the PE's float32r rounding (measured)

TRACE = False
LAST_EXEC_NS = None
LAST_RESULTS = None

_NC_CACHE = {}


def round_mant(x, bits=F32R_BITS):
    """Round fp32 to `bits` explicit mantissa bits (round-to-nearest-even).

    Matches TRN2's float32r rounding (verified on hardware for bits=11)."""
    x = np.ascontiguousarray(x, np.float32)
    xi = x.view(np.uint32)
    shift = 23 - bits
    unit = np.uint32(1 << shift)
    half = np.uint32(1 << (shift - 1))
    low = xi & np.uint32(unit - 1)
    xi2 = xi & ~np.uint32(unit - 1)
    rup = (low > half) | ((low == half) & ((xi2 >> np.uint32(shift)) & 1).astype(bool))
    xi2 = xi2 + np.where(rup, unit, np.uint32(0))
    return xi2.view(np.float32)


def emit_attention(nc, tc, ins, outs, pairs, s, d):
    """Emit the per-core attention program.

    ins:  qhc [pairs, 128, s]  bf16, rows 0..63 = Qlo^T, rows 64..127 = Qhi^T
          qhi [pairs, 64, s]   bf16 Qhi^T again at base partition 0 (pass-1
                               rhs; matmul requires equal base partitions)
          khc [pairs, 128, s]  bf16, rows 0..63 = Khi^T, rows 64..127 = Klo^T
          vm  [pairs, s, 65]   V (pre-rounded to f32r) + ones column
          maskb [pairs, s]     additive mask bias (0 or -1e9) per key
    outs: pt [pairs, s, s] (= P^T, [k, q]) and ot [pairs, d, s] (= out^T).
    """
    import concourse.bass as bass
    import concourse.mybir as mybir
    from contextlib import ExitStack

    f32 = mybir.dt.float32
    f32r = mybir.dt.float32r
    bf16 = mybir.dt.bfloat16
    vr = d + 1
    kb_n = s // 128  # k blocks
    qh_w = s // 2  # process q in two halves (PSUM budget)
    n512 = (qh_w + 511) // 512

    qhc_d, qhi_d, khc_d, vm_d = ins["qhc"], ins["qhi"], ins["khc"], ins["vm"]
    maskb_d = ins["maskb"]
    pt_d, ot_d = outs["pt"], outs["ot"]

    with ExitStack() as ctx:
        sb = ctx.enter_context(tc.tile_pool(name="sb", bufs=1))
        raw = ctx.enter_context(tc.tile_pool(name="raw", bufs=1))
        aux = ctx.enter_context(tc.tile_pool(name="aux", bufs=2))
        punt_pool = ctx.enter_context(tc.tile_pool(name="punt", bufs=2 * kb_n + 2))
        stp = ctx.enter_context(tc.tile_pool(name="stp", bufs=2, space="PSUM"))
        outp = ctx.enter_context(tc.tile_pool(name="outp", bufs=1, space="PSUM"))

        def load_pair(p):
            # Q/K arrive as bf16 hi/lo stacks -> straight DMA, no rounding op
            # needed (the f32r verifier rule applies only to f32r operands).
            qt = sb.tile([128, s], bf16, tag="qt", name=f"qt_{p}", bufs=2)
            nc.sync.dma_start(out=qt, in_=qhc_d[p])
            qht = sb.tile([64, s], bf16, tag="qhi", name=f"qhi_{p}", bufs=2)
            nc.sync.dma_start(out=qht, in_=qhi_d[p])
            kt = sb.tile([128, s], bf16, tag="kt", name=f"kt_{p}", bufs=2)
            nc.sync.dma_start(out=kt, in_=khc_d[p])
            # V arrives pre-rounded to f32r values; the round copy is a
            # numeric identity but must exist: the BIR verifier requires every
            # f32r matmul operand to be produced by a rounding-capable op.
            vraw = raw.tile([128, kb_n, vr], f32, tag="raw", name=f"vraw_{p}")
            nc.sync.dma_start(
                out=vraw, in_=vm_d[p].rearrange("(n pp) c -> pp n c", pp=128)
            )
            vt = sb.tile([128, kb_n, vr], f32r, tag="v", name=f"v_{p}", bufs=1)
            nc.gpsimd.tensor_copy(out=vt, in_=vraw)
            maskt = sb.tile([128, kb_n], f32, tag="mask", name=f"mask_{p}", bufs=2)
            nc.sync.dma_start(
                out=maskt, in_=maskb_d[p].rearrange("(n pp) -> pp n", pp=128)
            )
            return qt, qht, kt, vt, maskt

        loaded = {0: load_pair(0)}
        for p in range(pairs):
            if p + 1 < pairs:
                loaded[p + 1] = load_pair(p + 1)
            qt, qht, kt, vt, maskt = loaded.pop(p)

            outT = outp.tile([vr, s], f32, tag="outT", name=f"outT_{p}")

            for qh in range(2):
                qlo, qhi_ = qh * qh_w, (qh + 1) * qh_w
                punts = [
                    punt_pool.tile(
                        [128, qh_w], f32r, tag="punt", name=f"pun_{p}_{qh}_{kb}"
                    )
                    for kb in range(kb_n)
                ]
                for kb in range(kb_n):
                    st = stp.tile([128, qh_w], f32, tag="st", name=f"st_{p}_{qh}_{kb}")
                    ks = slice(kb * 128, (kb + 1) * 128)
                    chunks = [
                        (c0, min(qh_w, c0 + 512)) for c0 in range(0, qh_w, 512)
                    ]
                    # QK pass 1: Khi . Qhi (K=64)
                    for c0, c1 in chunks:
                        nc.tensor.matmul(
                            st[:, c0:c1],
                            kt[0:64, ks],
                            qht[:, qlo + c0 : qlo + c1],
                            start=True,
                            stop=False,
                        )
                    # QK pass 2: Khi.Qlo + Klo.Qhi (K=128)
                    for c0, c1 in chunks:
                        nc.tensor.matmul(
                            st[:, c0:c1],
                            kt[:, ks],
                            qt[:, qlo + c0 : qlo + c1],
                            start=False,
                            stop=True,
                        )
                    nc.scalar.activation(
                        punts[kb][:, :],
                        st[:, :],
                        mybir.ActivationFunctionType.Exp,
                        bias=maskt[:, kb : kb + 1],
                        scale=0.125,
                    )
                    for c0, c1 in chunks:
                        nc.tensor.matmul(
                            outT[:, qlo + c0 : qlo + c1],
                            vt[:, kb, :],
                            punts[kb][:, c0:c1],
                            start=(kb == 0),
                            stop=(kb == kb_n - 1),
                        )

                # --- per-q-half tail: recip of rowsum, normalize, store ---
                # (outT[:, qlo:qhi_] is complete once this qh's kb loop ends,
                # so the tail overlaps the other q-half's compute)
                # reciprocal of rowsum, entirely in SBUF:
                #   [1, qh_w] row -> [128, qh_w/128] (SBUF-SBUF DMA reshape,
                #   so DVE reciprocal runs on all 128 lanes) -> back to a
                #   [1, qh_w] row -> stride-0 partition-replicate to [128, qh_w]
                rs_sb = aux.tile([1, qh_w], f32, tag="aux", name=f"rs_{p}_{qh}")
                nc.vector.tensor_copy(out=rs_sb, in_=outT[d : d + 1, qlo:qhi_])
                rs128 = sb.tile(
                    [128, qh_w // 128], f32, tag="rs128", name=f"rs128_{p}_{qh}"
                )
                rs_split = bass.AP(
                    tensor=rs_sb.tensor,
                    offset=rs_sb.offset,
                    ap=[[1, 1], [qh_w // 128, 128], [1, qh_w // 128]],
                )
                nc.sync.dma_start(out=rs128, in_=rs_split)
                rec128 = sb.tile(
                    [128, qh_w // 128], f32, tag="rec128", name=f"rec128_{p}_{qh}"
                )
                nc.vector.reciprocal(out=rec128, in_=rs128)
                rec_row = aux.tile([1, qh_w], f32, tag="aux", name=f"rr_{p}_{qh}")
                rr_split = bass.AP(
                    tensor=rec_row.tensor,
                    offset=rec_row.offset,
                    ap=[[1, 1], [qh_w // 128, 128], [1, qh_w // 128]],
                )
                nc.sync.dma_start(out=rr_split, in_=rec128)
                rb = sb.tile([128, qh_w], f32, tag="rb", name=f"rb_{p}_{qh}", bufs=2)
                nc.gpsimd.partition_broadcast(rb, rec_row)

                oTs = aux.tile([d, qh_w], f32, tag="aux", name=f"oTs_{p}_{qh}")
                nc.vector.tensor_copy(out=oTs, in_=outT[0:d, qlo:qhi_])
                nc.vector.tensor_mul(out=oTs, in0=oTs, in1=rb[0:d, :])
                nc.sync.dma_start(out=ot_d[p, :, qlo:qhi_], in_=oTs)

                # normalize P^T in place; f32r output dtype keeps the BIR
                # verifier happy (PV matmuls consume this tile); costs one
                # extra 11-bit rounding on the stored p_attn.
                for kb in range(kb_n):
                    pn = punts[kb]
                    nc.vector.tensor_mul(out=pn, in0=pn, in1=rb)
                    nc.sync.dma_start(
                        out=pt_d[p, kb * 128 : (kb + 1) * 128, qlo:qhi_],
                        in_=pn.bitcast(f32),
                    )


def _build_nc(pairs=PAIRS, s=S, d=D):
    import concourse.bacc as bacc
    import concourse.mybir as mybir
    from concourse import tile

    key = (pairs, s, d)
    if key in _NC_CACHE:
        return _NC_CACHE[key]

    f32 = mybir.dt.float32
    vr = d + 1
    nc = bacc.Bacc(
        "TRN2",
        target_bir_lowering=False,
        debug=False,
        enable_asserts=False,
        num_devices=NCORES,
    )
    bf16 = mybir.dt.bfloat16
    ins = {
        "qhc": nc.dram_tensor("qhc", [pairs, 128, s], bf16, kind="ExternalInput").ap(),
        "qhi": nc.dram_tensor("qhi", [pairs, 64, s], bf16, kind="ExternalInput").ap(),
        "khc": nc.dram_tensor("khc", [pairs, 128, s], bf16, kind="ExternalInput").ap(),
        "vm": nc.dram_tensor("vm", [pairs, s, vr], f32, kind="ExternalInput").ap(),
        "maskb": nc.dram_tensor("maskb", [pairs, s], f32, kind="ExternalInput").ap(),
    }
    outs = {
        "pt": nc.dram_tensor("pt", [pairs, s, s], f32, kind="ExternalOutput").ap(),
        "ot": nc.dram_tensor("ot", [pairs, d, s], f32, kind="ExternalOutput").ap(),
    }
    with tile.TileContext(nc) as tc:
        emit_attention(nc, tc, ins, outs, pairs, s, d)
    nc.compile()
    _NC_CACHE[key] = nc
    return nc


def prep_inputs(query, key, value, mask):
    """Full inputs -> per-core in_maps (list of 8 dicts)."""
    q = np.ascontiguousarray(np.asarray(query, np.float32)).reshape(B * H, S, D)
    k = np.ascontiguousarray(np.asarray(key, np.float32)).reshape(B * H, S, D)
    v = np.ascontiguousarray(np.asarray(value, np.float32)).reshape(B * H, S, D)
    m = np.asarray(mask)

    import ml_dtypes

    bf16 = ml_dtypes.bfloat16
    qT = q.transpose(0, 2, 1)  # [24, 64, S]
    kT = k.transpose(0, 2, 1)
    qhi = qT.astype(bf16)
    qlo = (qT - qhi.astype(np.float32)).astype(bf16)
    khi = kT.astype(bf16)
    klo = (kT - khi.astype(np.float32)).astype(bf16)
    qhc = np.concatenate([qlo, qhi], axis=1)  # [24, 128, S] bf16
    khc = np.concatenate([khi, klo], axis=1)  # [24, 128, S] bf16
    vm = np.concatenate(
        [round_mant(v), np.ones((B * H, S, 1), np.float32)], axis=2
    )  # [24, S, 65]
    maskb = np.where(m == 0, np.float32(-1e9), np.float32(0.0)).astype(np.float32)
    maskb = np.repeat(maskb[:, None, :], H, axis=1).reshape(B * H, S)

    in_maps = []
    for c in range(NCORES):
        sl = slice(c * PAIRS, (c + 1) * PAIRS)
        in_maps.append(
            {
                "qhc": np.ascontiguousarray(qhc[sl]),
                "qhi": np.ascontiguousarray(qhi[sl]),
                "khc": np.ascontiguousarray(khc[sl]),
                "vm": np.ascontiguousarray(vm[sl]),
                "maskb": np.ascontiguousarray(maskb[sl]),
            }
        )
    return in_maps


def kernel(query, key, value, mask):
    global LAST_EXEC_NS, LAST_RESULTS
    from concourse.bass_utils import run_bass_kernel_spmd

    nc = _build_nc()
    in_maps = prep_inputs(query, key, value, mask)
    res = run_bass_kernel_spmd(
        nc,
        in_maps,
        core_ids=list(range(NCORES)),
        trace=TRACE,
    )
    LAST_EXEC_NS = res.exec_time_ns
    LAST_RESULTS = res
    pt = np.concatenate([r["pt"] for r in res.results], axis=0)  # [24, S(k), S(q)]
    ot = np.concatenate([r["ot"] for r in res.results], axis=0)  # [24, D, S]
    p_attn = np.ascontiguousarray(pt.reshape(B, H, S, S).swapaxes(2, 3))
    out = np.ascontiguousarray(ot.reshape(B, H, D, S).swapaxes(2, 3))
    return out, p_attn
